# revision 8
# baseline (speedup 1.0000x reference)
"""2-layer GCN encoder as a distributed Bass kernel on 8 TRN2 NeuronCores.

Decomposition (per core, nodes sharded by destination):
  hs1[v]  = dinv[v] * (x[v] @ W1)                 (own rows, bf16)
  S1T[:,d]= sum_{e: dst=d} hs1[src_e]             (dma_gather + one-hot matmul,
                                                   accumulated TRANSPOSED)
  hsrT    = relu(dinv_col * S1T + b1)             (dst-side dinv per column)
  hsr2[v] = dinv[v] * (hsrT^T @ W2)               (W2 commutes with the layer-2
                                                   edge sum -> aggregate at 64)
  S2[d]   = sum_{e: dst=d} hsr2[src_e]
  y[d]    = dinv[d]*S2[d] + b2

Source nodes are split into NG=2 groups (halves of each core's row range) so
each layer's AllGather runs as two chunked collectives that overlap with the
SWDGE gather stream (the critical resource: ~2ns of Q7 descriptor-emission
time per gathered row).  Destination blocks are processed in two halves of 15
so layer-2's first collective fires at the midpoint of layer 1's consumption.

Gather instructions pack chunks across block boundaries (up to MAXCH chunks,
single_packet=False which is required above 64 descriptors per engine).
Trailing pad indices are -1 (the Q7 ucode trims them per core); interior pads
are 0 and their one-hot rows are zero.
"""

import numpy as np

import concourse.bass as bass
import concourse.bacc as bacc
import concourse.mybir as mybir
import concourse.tile as tile
from concourse import library_config
from concourse.bass_utils import run_bass_kernel_spmd

F32 = mybir.dt.float32
BF16 = mybir.dt.bfloat16
FP8 = mybir.dt.float8e4
I16 = mybir.dt.int16

NCORES = 8
BLK = 128
MAXCH = 16     # chunks (128 idx each) per dma_gather instruction
NQUEUES = 4
NG = 2         # source groups (chunked AllGathers)


def _cdiv(a, b):
    return (a + b - 1) // b


def preprocess(x, edge_index, ncores=NCORES):
    """Host-side graph partitioning: shard nodes/edges by dst, split sources
    into NG owner-row groups, build per-core gather indices (group-row ids,
    SWDGE wrapped layout) and the one-hot chunk matrices (fp8)."""
    import ml_dtypes

    N, IN = x.shape
    assert N % ncores == 0
    NP = N // ncores
    nblk = _cdiv(NP, BLK)
    R = 6                              # blocks per processing round
    NR = _cdiv(nblk, R)
    GSPLIT = 18                        # source-group boundary (blocks)
    gb = [0, GSPLIT * BLK, NP]         # source-group boundaries (owner offset)
    NPg = [gb[1] - gb[0], gb[2] - gb[1]]
    widths = [min(BLK, NP - b * BLK) for b in range(nblk)]
    rounds = [list(range(r * R, min((r + 1) * R, nblk))) for r in range(NR)]

    src = np.asarray(edge_index[0], dtype=np.int64)
    dst = np.asarray(edge_index[1], dtype=np.int64)
    deg = (np.bincount(dst, minlength=N) + 1).astype(np.float32)

    # dedupe repeated (src, dst) pairs; multiplicity goes into the multi-hot
    key = dst * N + src
    ukey, mult = np.unique(key, return_counts=True)
    dst_s = ukey // N
    src_s = ukey % N
    mult = mult.astype(np.float32)

    # source group + row within the group's AllGather output (rank-major)
    srcr = src_s // NP
    srco = src_s % NP
    sg = (srco >= gb[1]).astype(np.int64)
    grow = srcr * np.asarray(NPg)[sg] + (srco - np.asarray(gb)[sg])

    bounds = np.array(
        [i * NP + b * BLK for i in range(ncores) for b in range(nblk)] + [N],
        dtype=np.int64,
    )
    pos = np.searchsorted(dst_s, bounds)

    # per (core, block, group): sorted unique group-rows + scatter triplets
    blk_rows = {}
    blk_scatter = {}
    ucnt = np.zeros((ncores, nblk, NG), np.int64)
    for i in range(ncores):
        for b in range(nblk):
            k = i * nblk + b
            s0, s1 = pos[k], pos[k + 1]
            dl = (dst_s[s0:s1] - (i * NP + b * BLK)).astype(np.int64)
            for g in range(NG):
                m = sg[s0:s1] == g
                urows, inv = np.unique(grow[s0:s1][m], return_inverse=True)
                ucnt[i, b, g] = len(urows)
                blk_rows[i, b, g] = urows
                blk_scatter[i, b, g] = (inv, dl[m], mult[s0:s1][m])

    CH = np.maximum(1, _cdiv(ucnt.max(axis=0), 128)).astype(np.int64)  # [b, g]

    # chunk stream order: for each round: for g in (0,1): blocks of the round
    cbase = {}
    c = 0
    stream_span = {}  # (r, g) -> (chunk_start, chunk_end)
    for r in range(NR):
        for g in range(NG):
            st = c
            for b in rounds[r]:
                cbase[b, g] = c
                c += int(CH[b, g])
            stream_span[r, g] = (st, c)
    NCHT = c

    # segments: pack MAXCH chunks within each stream
    segments = []  # (r, g, c0, sch)
    for r in range(NR):
        for g in range(NG):
            st, en = stream_span[r, g]
            for c0 in range(st, en, MAXCH):
                segments.append((r, g, c0, min(MAXCH, en - c0)))

    # first pass: raw index values per core (-1 where no real source)
    all_vals = []
    all_ohs = []
    for i in range(ncores):
        vals = np.full(NCHT * 128, -1, np.int64)
        ohs = np.zeros((128, NCHT * 128), np.float32)
        for b in range(nblk):
            for g in range(NG):
                urows = blk_rows[i, b, g]
                inv, dl, mlt = blk_scatter[i, b, g]
                c0 = cbase[b, g]
                vals[c0 * 128 : c0 * 128 + len(urows)] = urows
                q = inv  # slot of each edge's unique source
                gc = c0 + q // 128
                p = q % 128
                np.add.at(ohs, (p, gc * 128 + dl), mlt)
        all_vals.append(vals)
        all_ohs.append(ohs)

    # per-segment valid count must be uniform across cores: num_idxs_reg is
    # baked into the shared program, and the ucode's trailing trim must land
    # exactly at the register value on every core (decode bookkeeps with the
    # register; the impl trims by value -- a mismatch wedges the rings)
    seg_valid = []
    for (r, g, c0, sch) in segments:
        nv = 1
        for i in range(ncores):
            v = all_vals[i][c0 * 128 : (c0 + sch) * 128]
            nz = np.nonzero(v >= 0)[0]
            if len(nz):
                nv = max(nv, int(nz[-1]) + 1)
        seg_valid.append(nv)

    per_core = []
    for i in range(ncores):
        vals = all_vals[i]
        ohs = all_ohs[i]
        gidx = np.zeros((128, NCHT * 8), np.int16)
        for si, (r, g, c0, sch) in enumerate(segments):
            v = vals[c0 * 128 : (c0 + sch) * 128].copy()
            nv = seg_valid[si]
            v[:nv][v[:nv] < 0] = 0     # interior pads gather row 0
            v[nv:] = -1                # uniform trailing trim point
            wr = v.reshape(sch * 8, 16).T  # [16, sch*8]
            gidx[:, c0 * 8 : (c0 + sch) * 8] = np.tile(wr.astype(np.int16), (8, 1))

        degp = np.concatenate(
            [deg[i * NP : (i + 1) * NP], np.ones(nblk * BLK - NP, np.float32)]
        )
        per_core.append(
            {
                "x_tr": np.ascontiguousarray(
                    x[i * NP : (i + 1) * NP].T.astype(ml_dtypes.bfloat16)
                ),
                "deg_own": np.ascontiguousarray(degp.reshape(nblk, BLK).T),
                "deg_row": np.ascontiguousarray(degp.reshape(1, nblk * BLK)),
                "gidx": gidx,
                "ohs": ohs.astype(ml_dtypes.float8_e4m3),
            }
        )

    meta = {
        "N": N,
        "NP": NP,
        "IN": IN,
        "nblk": nblk,
        "R": R,
        "NR": NR,
        "GSPLIT": GSPLIT,
        "NPg": NPg,
        "rounds": rounds,
        "widths": widths,
        "CH": CH,
        "cbase": cbase,
        "segments": segments,
        "seg_valid": seg_valid,
        "NCHT": NCHT,
    }
    return per_core, meta


def build_nc(meta, HID, OUT, ncores=NCORES):
    N, NP, IN = meta["N"], meta["NP"], meta["IN"]
    nblk, widths = meta["nblk"], meta["widths"]
    R, NR, NPg, rounds = meta["R"], meta["NR"], meta["NPg"], meta["rounds"]
    GSPLIT = meta["GSPLIT"]
    CH, cbase, segments, NCHT = meta["CH"], meta["cbase"], meta["segments"], meta["NCHT"]
    seg_valid = meta["seg_valid"]
    KC = IN // 128
    assert IN % 128 == 0 and HID == 128 and OUT <= 128

    nc = bacc.Bacc(
        "TRN2",
        target_bir_lowering=False,
        debug=False,
        num_devices=ncores,
        num_swdge_queues=NQUEUES,
    )

    x_tr = nc.dram_tensor("x_tr", [IN, NP], BF16, kind="ExternalInput")
    w1 = nc.dram_tensor("w1", [IN, HID], BF16, kind="ExternalInput")
    b1c = nc.dram_tensor("b1c", [HID, 1], F32, kind="ExternalInput")
    w2 = nc.dram_tensor("w2", [HID, OUT], F32, kind="ExternalInput")
    b2 = nc.dram_tensor("b2", [1, OUT], F32, kind="ExternalInput")
    deg_own = nc.dram_tensor("deg_own", [128, nblk], F32, kind="ExternalInput")
    deg_row = nc.dram_tensor("deg_row", [1, nblk * BLK], F32, kind="ExternalInput")
    gidx_d = nc.dram_tensor("gidx", [128, NCHT * 8], I16, kind="ExternalInput")
    ident_d = nc.dram_tensor("ident", [128, 128], BF16, kind="ExternalInput")
    ohs_d = nc.dram_tensor("ohs", [128, NCHT * 128], FP8, kind="ExternalInput")
    y = nc.dram_tensor("y", [NP, OUT], F32, kind="ExternalOutput")

    hs1_stage = nc.dram_tensor("hs1_stage", [NP, HID], BF16)
    hsr2_stage = nc.dram_tensor("hsr2_stage", [NP, 128], BF16)
    hs1_full = [
        nc.dram_tensor(f"hs1_full{g}", [ncores * NPg[g], HID], BF16,
                       addr_space="Shared")
        for g in range(NG)
    ]
    hsr2_full = [
        nc.dram_tensor(f"hsr2_full{g}", [ncores * NPg[g], 128], BF16,
                       addr_space="Shared")
        for g in range(NG)
    ]
    rg = [list(range(ncores))]
    qn = [0]

    def next_q():
        q = qn[0]
        qn[0] = (q + 1) % NQUEUES
        return q

    # last chunk of each block (end of its g1 stream) -> stop flag
    lastc = {b: cbase[b, NG - 1] + int(CH[b, NG - 1]) - 1 for b in range(nblk)}
    # chunk -> (block, width) lookup
    chunk_blk = {}
    for b in range(nblk):
        for g in range(NG):
            for cc in range(int(CH[b, g])):
                chunk_blk[cbase[b, g] + cc] = b

    XG = 10  # blocks per x-load slice

    with tile.TileContext(nc) as tc:
        with (
            tc.tile_pool(name="const", bufs=1) as constp,
            tc.tile_pool(name="gath", bufs=10) as gathp,
            tc.tile_pool(name="hs", bufs=4) as hsp,
        ):
            nc.gpsimd.load_library(library_config.mlp)

            # ---- loads needed by phase B ----
            xsb = {}
            for k in range(KC):
                for s in range(0, nblk, XG):
                    cols = sum(widths[s : s + XG])
                    t = constp.tile([128, XG * BLK], BF16, tag=f"x{k}_{s}")
                    nc.sync.dma_start(
                        out=t[:, :cols],
                        in_=x_tr[k * 128 : (k + 1) * 128,
                                 s * BLK : s * BLK + cols],
                    )
                    xsb[k, s] = t
            w1c = []
            for k in range(KC):
                t = constp.tile([128, HID], BF16, tag=f"w1c{k}")
                nc.sync.dma_start(out=t[:], in_=w1[k * 128 : (k + 1) * 128, :])
                w1c.append(t)
            dinv_sb = constp.tile([128, nblk], F32, tag="dinv")
            nc.sync.dma_start(out=dinv_sb[:], in_=deg_own[:, :])
            nc.scalar.sqrt(dinv_sb[:], dinv_sb[:])
            nc.vector.reciprocal(dinv_sb[:], dinv_sb[:])

            # ---- phase B + broadcasts (own PSUM scope) ----
            hs1_t = []
            with tc.tile_pool(name="psB", bufs=2, space="PSUM") as psB:
                for b in range(nblk):
                    w = widths[b]
                    ph = psB.tile([128, HID], F32, tag="acc")
                    for k in range(KC):
                        nc.tensor.matmul(
                            ph[:w, :],
                            lhsT=xsb[k, (b // XG) * XG][:, (b % XG) * BLK : (b % XG) * BLK + w],
                            rhs=w1c[k][:, :],
                            start=(k == 0),
                            stop=(k == KC - 1),
                        )
                    t = constp.tile([128, HID], BF16, tag=f"hs1_{b}")
                    nc.scalar.activation(
                        t[:w, :],
                        ph[:w, :],
                        mybir.ActivationFunctionType.Copy,
                        scale=dinv_sb[:w, b : b + 1],
                    )
                    nc.sync.dma_start(
                        out=hs1_stage[b * BLK : b * BLK + w, :], in_=t[:w, :]
                    )
                    hs1_t.append(t)
                    if b == GSPLIT - 1:
                        nc.gpsimd.collective_compute(
                            "AllGather",
                            mybir.AluOpType.bypass,
                            replica_groups=rg,
                            ins=[hs1_stage[0 : NPg[0], :].opt()],
                            outs=[hs1_full[0][0 : ncores * NPg[0], :].opt()],
                        )
                nc.gpsimd.collective_compute(
                    "AllGather",
                    mybir.AluOpType.bypass,
                    replica_groups=rg,
                    ins=[hs1_stage[NPg[0] : NP, :].opt()],
                    outs=[hs1_full[1][0 : ncores * NPg[1], :].opt()],
                )

                # remaining constants (overlap the AllGathers / barrier)
                gidx_sb = constp.tile([128, NCHT * 8], I16, tag="gidx")
                nc.sync.dma_start(out=gidx_sb[:], in_=gidx_d[:, :])
                ident_sb = constp.tile([128, 128], BF16, tag="ident")
                nc.sync.dma_start(out=ident_sb[:], in_=ident_d[:, :])
                w2_sb = constp.tile([HID, OUT], F32, tag="w2")
                nc.sync.dma_start(out=w2_sb[:], in_=w2[:, :])
                b1_sb = constp.tile([HID, 1], F32, tag="b1")
                nc.sync.dma_start(out=b1_sb[:], in_=b1c[:, :])
                b2_sb = constp.tile([1, OUT], F32, tag="b2")
                nc.sync.dma_start(out=b2_sb[:], in_=b2[:, :])
                ones_sb = constp.tile([1, 128], F32, tag="ones")
                nc.vector.memset(ones_sb[:], 1.0)
                ohs_sb = constp.tile([128, NCHT * 128], FP8, tag="ohs")
                qcols = _cdiv(NCHT * 128, 4)
                for qq in range(4):
                    c0q = qq * qcols
                    c1q = min((qq + 1) * qcols, NCHT * 128)
                    nc.sync.dma_start(out=ohs_sb[:, c0q:c1q], in_=ohs_d[:, c0q:c1q])

                # broadcast b2 to all partitions via rank-1 matmul
                pb2 = psB.tile([128, 128], F32, tag="aux")
                nc.tensor.matmul(pb2[:, :OUT], lhsT=ones_sb[:], rhs=b2_sb[:],
                                 start=True, stop=True)
                b2_bc = constp.tile([128, OUT], F32, tag="b2bc")
                nc.vector.tensor_copy(b2_bc[:], pb2[:, :OUT])

                # per-column dinv for the transposed layer-1 epilogue
                deg_rsb = constp.tile([1, nblk * BLK], F32, tag="degrow")
                nc.sync.dma_start(out=deg_rsb[:], in_=deg_row[:, :])
                dinv_bc = constp.tile([128, nblk * BLK], F32, tag="dinvbc")
                for b in range(nblk):
                    pdv = psB.tile([128, 128], F32, tag="aux")
                    nc.tensor.matmul(
                        pdv[:], lhsT=ones_sb[:],
                        rhs=deg_rsb[:, b * BLK : (b + 1) * BLK],
                        start=True, stop=True,
                    )
                    nc.vector.tensor_copy(dinv_bc[:, b * BLK : (b + 1) * BLK], pdv[:])
                nc.scalar.sqrt(dinv_bc[:], dinv_bc[:])
                nc.vector.reciprocal(dinv_bc[:], dinv_bc[:])

            # zero the gather pool once: trailing-trimmed lanes expose stale
            # SBUF; first use must not contain NaN-decoding garbage
            for zi in range(10):
                zt = gathp.tile([128, MAXCH, HID], BF16, tag="g", name=f"z{zi}")
                nc.vector.memset(zt[:, :, :], 0.0)

            segs_of = {}
            for si, (r, g, c0, sch) in enumerate(segments):
                segs_of.setdefault((r, g), []).append((c0, sch, seg_valid[si]))

            def gather_and_mm(layer, r, g, acc, table, elem):
                """Issue one stream's gathers; matmuls consume each segment."""
                for (c0, sch, nv) in segs_of[r, g]:
                    t = gathp.tile([128, MAXCH, HID], BF16, tag="g")
                    nc.gpsimd.dma_gather(
                        t[:, :sch, :],
                        table.ap(),
                        gidx_sb[:, c0 * 8 : (c0 + sch) * 8],
                        sch * 128,
                        nv,
                        elem,
                        queue_num=next_q(),
                        single_packet=False,
                    )
                    for cc in range(sch):
                        gc = c0 + cc
                        b = chunk_blk[gc]
                        w = widths[b]
                        if layer == 1:
                            nc.tensor.matmul(
                                acc[b][:, :w],
                                lhsT=t[:, cc, :],
                                rhs=ohs_sb[:, gc * 128 : gc * 128 + w],
                                start=False,
                                stop=(gc == lastc[b]),
                            )
                        else:
                            nc.tensor.matmul(
                                acc[b][:w, :OUT],
                                lhsT=ohs_sb[:, gc * 128 : gc * 128 + w],
                                rhs=t[:, cc, :OUT],
                                start=False,
                                stop=(gc == lastc[b]),
                            )

            # ---- layer 1: S1^T -> hsr^T -> hsr2 ----
            hsr2_t = [None] * nblk
            with tc.tile_pool(name="psD", bufs=1, space="PSUM") as psD:
                p2s_bufs = [
                    psD.tile([128, 128], F32, tag=f"p2s{i}", name=f"p2s{i}")
                    for i in range(2)
                ]
                for r in range(NR):
                    acc = {}
                    for b in rounds[r]:
                        acc[b] = psD.tile(
                            [128, 128], F32, tag=f"accD{b - r * R}",
                            name=f"accD{b}",
                        )
                        # self-loop opens the accumulation group
                        w = widths[b]
                        nc.tensor.matmul(
                            acc[b][:, :w], lhsT=hs1_t[b][:w, :],
                            rhs=ident_sb[:w, :w],
                            start=True, stop=False,
                        )
                    for g in range(NG):
                        gather_and_mm(1, r, g, acc, hs1_full[g], HID)
                    for bi, b in enumerate(rounds[r]):
                        w = widths[b]
                        t1 = hsp.tile([128, 128], F32, tag="t1")
                        nc.vector.tensor_tensor(
                            out=t1[:, :w], in0=acc[b][:, :w],
                            in1=dinv_bc[:, b * BLK : b * BLK + w],
                            op=mybir.AluOpType.mult,
                        )
                        hsrT = hsp.tile([128, 128], F32, tag="hsrT")
                        nc.scalar.activation(
                            hsrT[:, :w], t1[:, :w],
                            mybir.ActivationFunctionType.Relu,
                            bias=b1_sb[:, 0:1],
                        )
                        p2s = p2s_bufs[bi % 2]
                        nc.tensor.matmul(
                            p2s[:w, :OUT], lhsT=hsrT[:, :w], rhs=w2_sb[:, :],
                            start=True, stop=True,
                        )
                        t2 = constp.tile([128, 128], BF16, tag=f"hsr2_{b}")
                        nc.vector.memset(t2[:, OUT:], 0.0)
                        nc.scalar.activation(
                            t2[:w, :OUT], p2s[:w, :OUT],
                            mybir.ActivationFunctionType.Copy,
                            scale=dinv_sb[:w, b : b + 1],
                        )
                        nc.sync.dma_start(
                            out=hsr2_stage[b * BLK : b * BLK + w, :], in_=t2[:w, :]
                        )
                        hsr2_t[b] = t2
                    if rounds[r][-1] == GSPLIT - 1:
                        nc.gpsimd.collective_compute(
                            "AllGather",
                            mybir.AluOpType.bypass,
                            replica_groups=rg,
                            ins=[hsr2_stage[0 : NPg[0], :].opt()],
                            outs=[hsr2_full[0][0 : ncores * NPg[0], :].opt()],
                        )
                nc.gpsimd.collective_compute(
                    "AllGather",
                    mybir.AluOpType.bypass,
                    replica_groups=rg,
                    ins=[hsr2_stage[NPg[0] : NP, :].opt()],
                    outs=[hsr2_full[1][0 : ncores * NPg[1], :].opt()],
                )

            # ---- layer 2: S2 -> y ----
            with tc.tile_pool(name="psF", bufs=1, space="PSUM") as psF:
                for r in range(NR):
                    acc = {}
                    for b in rounds[r]:
                        acc[b] = psF.tile(
                            [128, 64], F32, tag=f"accF{b - r * R}",
                            name=f"accF{b}",
                        )
                        w = widths[b]
                        nc.tensor.matmul(
                            acc[b][:w, :OUT], lhsT=ident_sb[:w, :w],
                            rhs=hsr2_t[b][:w, :OUT],
                            start=True, stop=False,
                        )
                    for g in range(NG):
                        gather_and_mm(2, r, g, acc, hsr2_full[g], 128)
                    for b in rounds[r]:
                        w = widths[b]
                        o1 = hsp.tile([128, OUT], F32, tag="o1")
                        nc.scalar.activation(
                            o1[:w, :], acc[b][:w, :OUT],
                            mybir.ActivationFunctionType.Copy,
                            scale=dinv_sb[:w, b : b + 1],
                        )
                        yt = hsp.tile([128, OUT], F32, tag="yt")
                        nc.vector.tensor_tensor(
                            out=yt[:w, :], in0=o1[:w, :], in1=b2_bc[:w, :],
                            op=mybir.AluOpType.add,
                        )
                        nc.sync.dma_start(out=y[b * BLK : b * BLK + w, :],
                                          in_=yt[:w, :])

    nc.compile()
    return nc


def _make_ident():
    import ml_dtypes

    return np.eye(128, dtype=np.float32).astype(ml_dtypes.bfloat16)


_IDENT = _make_ident()


def make_in_maps(per_core, W1, b1, W2, b2):
    import ml_dtypes

    W1 = np.ascontiguousarray(np.asarray(W1, np.float32).astype(ml_dtypes.bfloat16))
    W2 = np.ascontiguousarray(np.asarray(W2, np.float32))
    b1 = np.ascontiguousarray(np.asarray(b1, np.float32).reshape(-1, 1))
    b2 = np.asarray(b2, np.float32).reshape(1, -1)
    return [
        {
            "x_tr": pc["x_tr"],
            "w1": W1,
            "b1c": b1,
            "w2": W2,
            "b2": b2,
            "deg_own": pc["deg_own"],
            "deg_row": pc["deg_row"],
            "gidx": pc["gidx"],
            "ohs": pc["ohs"],
            "ident": _IDENT,
        }
        for pc in per_core
    ]


def kernel_run(x, edge_index, W1, b1, W2, b2, trace=False, tmpdir=None):
    x = np.ascontiguousarray(np.asarray(x, np.float32))
    per_core, meta = preprocess(x, edge_index)
    HID = np.asarray(W1).shape[1]
    OUT = np.asarray(W2).shape[1]
    nc = build_nc(meta, HID, OUT)
    in_maps = make_in_maps(per_core, W1, b1, W2, b2)
    res = run_bass_kernel_spmd(
        nc, in_maps, core_ids=list(range(NCORES)), trace=trace, tmpdir=tmpdir
    )
    out = np.concatenate([r["y"] for r in res.results], axis=0)
    return out, res


def kernel(x, edge_index, W1, b1, W2, b2):
    out, _ = kernel_run(x, edge_index, W1, b1, W2, b2)
    return out
